# revision 1
# baseline (speedup 1.0000x reference)
"""MoE FFN (top-2 routing) for 8 Trainium2 NeuronCores.

Strategy: expert-parallel (core e owns expert e), exploiting top-2
sparsity. Only the tokens actually routed to an expert are computed --
numerically identical to the reference's dense masked-accumulate, since
zero-dispatch experts contribute exactly 0.

  - Router on host (0.01% of FLOPs): softmax(x@Wr), top-2, renormalize
    -> dispatch[B*T, E]. (Identical top-2 selection to jax on the
    reference input; min p2-p3 margin is 6.6e-6 >> fp32 noise.)
  - Host gathers each expert's tokens, padded to CAP=2304 slots
    (real per-expert loads: 1932..2182). Any overflow falls back to the
    dense all-tokens kernel (KERNEL_MODE=dense forces it).
  - Each core runs a two-phase fp32 FFN over its gathered tokens:
      Phase 1 (fc1): lhsT = W1 tile [d_k=128, h=128] (stationary),
                     rhs  = xT tile [d_k=128, m<=512] (moving),
                     psum [h=128, m] accumulated over 8 d_k tiles,
                     gelu(+b1 per-partition bias) evict -> hT DRAM.
      Phase 2 (fc2): lhsT = hT tile [h_k=128, m=128] (stationary),
                     rhs  = W2 tile [h_k=128, d=512] (moving),
                     psum [m=128, d=512] accumulated over 32 h_k tiles,
                     * dispatch weight (per-partition scalar) -> out.
  - Host scatter-adds the per-core partials (the "all-reduce") and adds
    b2 (sum_e disp_e = 1 after renormalization).

Device layout convention: a logical [R, C] matrix is stored in DRAM as
[128, R/128, C] with row r -> [r % 128, r // 128, :] (partition-inner).
W1/W2 stay fully resident in SBUF during their phase; the first x / hT
tiles are issued ahead of the 16MB weight DMA so the PE starts ~10us in
(cost model: 99% PE occupancy, one 10us gap at the phase boundary).

Matmul dtype is fp32 (bit-exact vs the reference computation, rel err
~9e-7 end to end). KERNEL_MM_DT=float32r selects TF32-like 11-bit
matmuls: ~2.6x faster (0.6-0.8ms vs ~2.0-2.5ms per exec), rel err 1.8e-4.
"""

import os
import sys
import numpy as np

if "/opt/trn_rl_repo" not in sys.path:
    sys.path.insert(0, "/opt/trn_rl_repo")

# Problem dims (hardcoded per contract).
B, T, D, H, E, TOPK = 2, 4096, 1024, 4096, 8, 2
M = B * T  # 8192 tokens
NCORES = 8
P = 128

_CACHE = {}
LAST_RESULTS = None

# Sparse path: per-expert token capacity (real input peaks at 2182).
CAP = 2304


def _route_host(x2, Wr):
    """Host router: returns dispatch [M, E] float32 (top-2 renormalized)."""
    logits = x2 @ Wr  # [M, E] fp32
    logits = logits - logits.max(axis=-1, keepdims=True)
    p = np.exp(logits)
    p = p / p.sum(axis=-1, keepdims=True)
    # top-2 of E=8
    a1 = np.argmax(p, axis=-1)
    rows = np.arange(p.shape[0])
    p1 = p[rows, a1]
    p_masked = p.copy()
    p_masked[rows, a1] = -np.inf
    a2 = np.argmax(p_masked, axis=-1)
    p2 = p_masked[rows, a2]
    s = p1 + p2
    disp = np.zeros_like(p)
    disp[rows, a1] = p1 / s
    disp[rows, a2] = p2 / s
    return disp.astype(np.float32)


def _pm(a2d):
    """[R, C] -> [128, R/128, C] with row r -> [r%128, r//128]."""
    R, C = a2d.shape
    return np.ascontiguousarray(a2d.reshape(R // P, P, C).transpose(1, 0, 2))


def _build_nc(mm_dt_name, M=M, reps=1):
    import concourse.bass as bass
    import concourse.bacc as bacc
    import concourse.mybir as mybir
    from concourse.tile import TileContext

    mm_dt = getattr(mybir.dt, mm_dt_name)
    f32 = mybir.dt.float32

    KD = D // P        # 8   d_k tiles
    KH = H // P        # 32  h_k tiles
    NHC = H // P       # 32  h chunks (phase 1)
    NMT = M // P       # token tiles (phase 2)
    NDB = D // 512     # 2   d blocks (phase 2)
    # Phase-1 token blocks: 512-wide plus an optional tail (multiple of 128)
    assert M % P == 0
    mb_sizes = [512] * (M // 512) + ([M % 512] if M % 512 else [])
    mb_offs = [sum(mb_sizes[:i]) for i in range(len(mb_sizes))]
    NMB = len(mb_sizes)
    # phase-2 tile -> (block, offset within block)
    mt_map = []
    for bi, (w, o) in enumerate(zip(mb_sizes, mb_offs)):
        for j in range(w // P):
            mt_map.append((bi, j * P))

    nc = bacc.Bacc(None, target_bir_lowering=False, debug=False)

    xT = nc.dram_tensor("xT", [P, KD, M], mm_dt, kind="ExternalInput")
    w1 = nc.dram_tensor("w1", [P, KD, H], mm_dt, kind="ExternalInput")
    w2 = nc.dram_tensor("w2", [P, KH, D], mm_dt, kind="ExternalInput")
    b1t = nc.dram_tensor("b1t", [P, NHC], f32, kind="ExternalInput")
    dsp = nc.dram_tensor("dsp", [P, NMT], f32, kind="ExternalInput")
    out = nc.dram_tensor("out", [P, NMT, D], f32, kind="ExternalOutput")

    with TileContext(nc) as tc:
        with tc.tile_pool(name="dram", bufs=1, space="DRAM") as dram, \
             tc.tile_pool(name="const", bufs=1) as const:
            # Intermediate hT, one DRAM tile per token block so phase 2
            # token tiles only depend on their own block's fc1 writes.
            hT_blocks = [
                dram.tile([P, NHC, mb_sizes[mb]], mm_dt, name=f"hT{mb}")
                for mb in range(NMB)
            ]
            b1_sb = const.tile([P, NHC], f32, name="b1_sb")
            nc.sync.dma_start(b1_sb[:], b1t[:])
            dsp_sb = const.tile([P, NMT], f32, name="dsp_sb")
            nc.sync.dma_start(dsp_sb[:], dsp[:])

            for rep in range(reps):
                # ---- Phase 1: hT = gelu(x @ W1 + b1), stored [H, M] ----
                with tc.tile_pool(name=f"w1p{rep}", bufs=1) as w1p, \
                     tc.tile_pool(name=f"xp{rep}", bufs=3) as xp, \
                     tc.tile_pool(name=f"hp{rep}", bufs=6) as hp, \
                     tc.tile_pool(name=f"ps1{rep}", bufs=4, space="PSUM") as ps1:
                    # First x block issued before the W1 chunk loads so the
                    # first matmul isn't queued behind 16MB of weight DMA.
                    x_first = xp.tile(
                        [P, KD, mb_sizes[0]], mm_dt, name="x_sb",
                        tag="x_sb")
                    nc.sync.dma_start(x_first[:], xT[:, :, 0:mb_sizes[0]])
                    # W1 resident, split per h-chunk for fine-grained deps.
                    w1_sb = []
                    for hc in range(NHC):
                        t = w1p.tile([P, KD, P], mm_dt, name=f"w1c{hc}")
                        nc.sync.dma_start(t[:], w1[:, :, hc * P:(hc + 1) * P])
                        w1_sb.append(t)
                    for mb in range(NMB):
                        w = mb_sizes[mb]
                        o = mb_offs[mb]
                        if mb == 0:
                            x_sb = x_first
                        else:
                            x_sb = xp.tile(
                                [P, KD, w], mm_dt, name="x_sb", tag="x_sb")
                            nc.sync.dma_start(x_sb[:], xT[:, :, o:o + w])
                        for hc in range(NHC):
                            psum = ps1.tile([P, w], f32, name="ps1t",
                                            tag="ps1t")
                            for k in range(KD):
                                nc.tensor.matmul(
                                    psum[:],
                                    lhsT=w1_sb[hc][:, k:k + 1, :],
                                    rhs=x_sb[:, k:k + 1, :],
                                    start=(k == 0),
                                    stop=(k == KD - 1),
                                )
                            h_sb = hp.tile([P, w], mm_dt, name="h_sb",
                                           tag="h_sb")
                            nc.scalar.activation(
                                h_sb[:], psum[:],
                                mybir.ActivationFunctionType.Gelu,
                                bias=b1_sb[:, hc:hc + 1],
                            )
                            nc.sync.dma_start(hT_blocks[mb][:, hc, :], h_sb[:])

                # ---- Phase 2: out = (hT.T @ W2) * disp ----
                with tc.tile_pool(name=f"w2p{rep}", bufs=1) as w2p, \
                     tc.tile_pool(name=f"hp2{rep}", bufs=3) as hp2, \
                     tc.tile_pool(name=f"op{rep}", bufs=6) as op, \
                     tc.tile_pool(name=f"ps2{rep}", bufs=4, space="PSUM") as ps2:
                    # First stationary tile issued before the W2 chunk loads
                    # so fc2's first matmul isn't queued behind 16MB of W2.
                    hT_first = hp2.tile([P, KH, P], mm_dt, name="hT_sb",
                                        tag="hT_sb")
                    nc.sync.dma_start(hT_first[:], hT_blocks[0][:, :, 0:P])
                    w2_sb = []
                    for k in range(KH):
                        t = w2p.tile([P, 1, D], mm_dt, name=f"w2c{k}")
                        nc.sync.dma_start(t[:], w2[:, k:k + 1, :])
                        w2_sb.append(t)
                    for mt in range(NMT):
                        mb, off = mt_map[mt]
                        if mt == 0:
                            hT_sb = hT_first
                        else:
                            hT_sb = hp2.tile([P, KH, P], mm_dt, name="hT_sb",
                                             tag="hT_sb")
                            nc.sync.dma_start(
                                hT_sb[:], hT_blocks[mb][:, :, off:off + P])
                        for db in range(NDB):
                            psum = ps2.tile([P, 512], f32, name="ps2t")
                            for k in range(KH):
                                nc.tensor.matmul(
                                    psum[:],
                                    lhsT=hT_sb[:, k:k + 1, :],
                                    rhs=w2_sb[k][:, :, db * 512:(db + 1) * 512],
                                    start=(k == 0),
                                    stop=(k == KH - 1),
                                )
                            o_sb = op.tile([P, 512], f32, name="o_sb")
                            nc.vector.tensor_scalar_mul(
                                o_sb[:], psum[:], dsp_sb[:, mt:mt + 1])
                            nc.sync.dma_start(
                                out[:, mt, db * 512:(db + 1) * 512], o_sb[:])

    nc.compile()
    return nc


def _get_nc(m_tokens=M):
    mm_dt_name = os.environ.get("KERNEL_MM_DT", "float32")
    key = ("nc", mm_dt_name, m_tokens)
    if key not in _CACHE:
        _CACHE[key] = _build_nc(mm_dt_name, M=m_tokens)
    return _CACHE[key]


class _Runner:
    """Cached jitted sharded invocation for one compiled Bass program."""

    def __init__(self, nc, n_cores):
        import jax
        from jax.sharding import Mesh, PartitionSpec
        from jax.experimental.shard_map import shard_map
        import concourse.mybir as mybir
        from concourse import bass2jax
        from concourse.bass2jax import _bass_exec_p, install_neuronx_cc_hook

        install_neuronx_cc_hook()
        self.jax = jax
        self.n_cores = n_cores
        partition_name = (
            nc.partition_id_tensor.name if nc.partition_id_tensor else None)
        in_names, out_names, out_avals = [], [], []
        for alloc in nc.m.functions[0].allocations:
            if not isinstance(alloc, mybir.MemoryLocationSet):
                continue
            name = alloc.memorylocations[0].name
            if alloc.kind == "ExternalInput":
                if name != partition_name:
                    in_names.append(name)
            elif alloc.kind == "ExternalOutput":
                out_names.append(name)
                out_avals.append(jax.core.ShapedArray(
                    tuple(alloc.tensor_shape), mybir.dt.np(alloc.dtype)))
        self.in_names = in_names
        self.out_names = out_names
        self.out_avals = out_avals
        n_params = len(in_names)
        n_outs = len(out_avals)
        all_in_names = in_names + out_names
        if partition_name is not None:
            all_in_names = all_in_names + [partition_name]

        def _body(*args):
            operands = list(args)
            if partition_name is not None:
                operands.append(bass2jax.partition_id_tensor())
            outs = _bass_exec_p.bind(
                *operands,
                out_avals=tuple(out_avals),
                in_names=tuple(all_in_names),
                out_names=tuple(out_names),
                lowering_input_output_aliases=(),
                sim_require_finite=True,
                sim_require_nnan=True,
                nc=nc,
            )
            return tuple(outs)

        devices = jax.devices()[:n_cores]
        mesh = Mesh(np.asarray(devices), ("core",))
        self.sh = jax.sharding.NamedSharding(mesh, PartitionSpec("core"))
        self.sharded = jax.jit(
            shard_map(_body, mesh=mesh,
                      in_specs=(PartitionSpec("core"),) * (n_params + n_outs),
                      out_specs=(PartitionSpec("core"),) * n_outs,
                      check_rep=False),
            donate_argnums=tuple(range(n_params, n_params + n_outs)),
            keep_unused=True)

    def put_inputs(self, in_maps):
        return [
            self.jax.device_put(
                np.concatenate(
                    [np.asarray(m[name]) for m in in_maps], axis=0), self.sh)
            for name in self.in_names
        ]

    def zeros(self):
        return [
            self.jax.device_put(
                np.zeros((self.n_cores * a.shape[0], *a.shape[1:]), a.dtype),
                self.sh)
            for a in self.out_avals
        ]

    def run(self, dev_in):
        out = self.sharded(*dev_in, *self.zeros())
        self.jax.block_until_ready(out)
        return out

    def to_results(self, out):
        return [
            {name: np.asarray(out[i]).reshape(
                self.n_cores, *self.out_avals[i].shape)[c]
             for i, name in enumerate(self.out_names)}
            for c in range(self.n_cores)
        ]


def _get_runner(nc):
    key = ("runner", id(nc))
    if key not in _CACHE:
        _CACHE[key] = _Runner(nc, NCORES)
    return _CACHE[key]


def bench_spmd(nc, in_maps, iters=5):
    """Time repeated on-device executions with device-resident inputs.
    Returns (best_seconds, results_of_last_call)."""
    import time as _time
    r = _get_runner(nc)
    dev_in = r.put_inputs(in_maps)
    out = r.run(dev_in)  # warmup (compiles once)
    best = float("inf")
    for _ in range(iters):
        z = r.zeros()
        r.jax.block_until_ready(z)
        t0 = _time.perf_counter()
        out = r.sharded(*dev_in, *z)
        r.jax.block_until_ready(out)
        best = min(best, _time.perf_counter() - t0)
    return best, r.to_results(out)


def _core_weight_inputs(W1, b1, W2, e):
    return {
        "w1": _pm(W1[e]),                       # [128, 8, 4096]
        "w2": _pm(W2[e]),                       # [128, 32, 1024]
        "b1t": np.ascontiguousarray(
            b1[e].reshape(H // P, P).T),        # [128, 32]
    }


def _dense_in_maps(x2, disp, W1, b1, W2):
    xT_pm = _pm(np.ascontiguousarray(x2.T))  # [128, 8, 8192]
    in_maps = []
    for e in range(NCORES):
        m = _core_weight_inputs(W1, b1, W2, e)
        m["xT"] = xT_pm
        m["dsp"] = np.ascontiguousarray(disp[:, e].reshape(M // P, P).T)
        in_maps.append(m)
    return in_maps


def _sparse_in_maps(x2, disp, W1, b1, W2):
    """Gather each expert's routed tokens (padded to CAP). Returns
    (in_maps, idx_list) or None if any expert overflows CAP."""
    in_maps, idx_list = [], []
    for e in range(NCORES):
        idx = np.nonzero(disp[:, e] > 0)[0]
        if idx.size > CAP:
            return None
        x_e = np.zeros((CAP, D), dtype=np.float32)
        x_e[:idx.size] = x2[idx]
        d_e = np.zeros((CAP,), dtype=np.float32)
        d_e[:idx.size] = disp[idx, e]
        m = _core_weight_inputs(W1, b1, W2, e)
        m["xT"] = _pm(np.ascontiguousarray(x_e.T))   # [128, 8, CAP]
        m["dsp"] = np.ascontiguousarray(d_e.reshape(CAP // P, P).T)
        in_maps.append(m)
        idx_list.append(idx)
    return in_maps, idx_list


def _run_spmd(nc, in_maps):
    r = _get_runner(nc)
    out = r.run(r.put_inputs(in_maps))
    return r.to_results(out)


def kernel(x, Wr, W1, b1, W2, b2):
    global LAST_RESULTS

    x2 = np.ascontiguousarray(np.asarray(x, dtype=np.float32).reshape(M, D))
    Wr = np.asarray(Wr, dtype=np.float32)
    W1 = np.asarray(W1, dtype=np.float32)
    b1 = np.asarray(b1, dtype=np.float32)
    W2 = np.asarray(W2, dtype=np.float32)
    b2 = np.asarray(b2, dtype=np.float32)

    disp = _route_host(x2, Wr)  # [M, E]
    mode = os.environ.get("KERNEL_MODE", "auto")

    sparse = None
    if mode in ("auto", "sparse"):
        sparse = _sparse_in_maps(x2, disp, W1, b1, W2)
    if sparse is not None:
        in_maps, idx_list = sparse
        nc = _get_nc(CAP)
        results = _run_spmd(nc, in_maps)
        LAST_RESULTS = results
        out2 = np.zeros((M, D), dtype=np.float32)
        for e in range(NCORES):
            y = results[e]["out"].transpose(1, 0, 2).reshape(CAP, D)
            out2[idx_list[e]] += y[:idx_list[e].size]
    else:
        in_maps = _dense_in_maps(x2, disp, W1, b1, W2)
        nc = _get_nc(M)
        results = _run_spmd(nc, in_maps)
        LAST_RESULTS = results
        acc = np.zeros((P, M // P, D), dtype=np.float32)
        for r in results:
            acc += r["out"]
        out2 = acc.transpose(1, 0, 2).reshape(M, D)

    out2 = out2 + disp @ b2  # sum_e disp_e * b2[e]
    return out2.reshape(B, T, D)



# revision 4
# speedup vs baseline: 11.7155x; 11.7155x over previous
"""MoE FFN (top-2 routing) for 8 Trainium2 NeuronCores.

Strategy: expert-parallel (core e owns expert e), exploiting top-2
sparsity. Only the tokens actually routed to an expert are computed --
numerically identical to the reference's dense masked-accumulate, since
zero-dispatch experts contribute exactly 0.

  - Router on host (0.01% of FLOPs): softmax(x@Wr), top-2, renormalize
    -> dispatch[B*T, E]. (Identical top-2 selection to jax on the
    reference input; min p2-p3 margin is 6.6e-6 >> fp32 noise.)
  - Host gathers each expert's tokens, padded to CAP=2304 slots
    (real per-expert loads: 1932..2182). Any overflow falls back to the
    dense all-tokens kernel (KERNEL_MODE=dense forces it).
  - Each core runs a two-phase fp32 FFN over its gathered tokens:
      Phase 1 (fc1): lhsT = W1 tile [d_k=128, h=128] (stationary),
                     rhs  = xT tile [d_k=128, m<=512] (moving),
                     psum [h=128, m] accumulated over 8 d_k tiles,
                     gelu(+b1 per-partition bias) evict -> hT DRAM.
      Phase 2 (fc2): lhsT = hT tile [h_k=128, m=128] (stationary),
                     rhs  = W2 tile [h_k=128, d=512] (moving),
                     psum [m=128, d=512] accumulated over 32 h_k tiles,
                     * dispatch weight (per-partition scalar) -> out.
  - Host scatter-adds the per-core partials (the "all-reduce") and adds
    b2 (sum_e disp_e = 1 after renormalization).

Device layout convention: a logical [R, C] matrix is stored in DRAM as
[128, R/128, C] with row r -> [r % 128, r // 128, :] (partition-inner).
W1/W2 stay fully resident in SBUF during their phase; the first x / hT
tiles are issued ahead of the 16MB weight DMA so the PE starts ~10us in
(cost model: 99% PE occupancy, one 10us gap at the phase boundary).

Matmul dtype is fp32 (bit-exact vs the reference computation, rel err
~9e-7 end to end). KERNEL_MM_DT=float32r selects TF32-like 11-bit
matmuls: ~2.6x faster (0.6-0.8ms vs ~2.0-2.5ms per exec), rel err 1.8e-4.
"""

import os
import sys
import numpy as np

if "/opt/trn_rl_repo" not in sys.path:
    sys.path.insert(0, "/opt/trn_rl_repo")

# Problem dims (hardcoded per contract).
B, T, D, H, E, TOPK = 2, 4096, 1024, 4096, 8, 2
M = B * T  # 8192 tokens
NCORES = 8
P = 128

_CACHE = {}
LAST_RESULTS = None

# Sparse path: per-expert token capacity (real input peaks at 2182).
CAP = 2304


def _route_host(x2, Wr):
    """Host router: returns dispatch [M, E] float32 (top-2 renormalized)."""
    logits = x2 @ Wr  # [M, E] fp32
    logits = logits - logits.max(axis=-1, keepdims=True)
    p = np.exp(logits)
    p = p / p.sum(axis=-1, keepdims=True)
    # top-2 of E=8
    a1 = np.argmax(p, axis=-1)
    rows = np.arange(p.shape[0])
    p1 = p[rows, a1]
    p_masked = p.copy()
    p_masked[rows, a1] = -np.inf
    a2 = np.argmax(p_masked, axis=-1)
    p2 = p_masked[rows, a2]
    s = p1 + p2
    disp = np.zeros_like(p)
    disp[rows, a1] = p1 / s
    disp[rows, a2] = p2 / s
    return disp.astype(np.float32)


def _pm(a2d):
    """[R, C] -> [128, R/128, C] with row r -> [r%128, r//128]."""
    R, C = a2d.shape
    return np.ascontiguousarray(a2d.reshape(R // P, P, C).transpose(1, 0, 2))


def _build_nc(mm_dt_name, M=M, reps=1):
    import concourse.bass as bass
    import concourse.bacc as bacc
    import concourse.mybir as mybir
    from concourse.tile import TileContext

    mm_dt = getattr(mybir.dt, mm_dt_name)
    f32 = mybir.dt.float32

    KD = D // P        # 8   d_k tiles
    KH = H // P        # 32  h_k tiles
    NHC = H // P       # 32  h chunks (phase 1)
    NMT = M // P       # token tiles (phase 2)
    NDB = D // 512     # 2   d blocks (phase 2)
    # Phase-1 token blocks: 512-wide plus an optional tail (multiple of 128)
    assert M % P == 0
    mb_sizes = [512] * (M // 512) + ([M % 512] if M % 512 else [])
    mb_offs = [sum(mb_sizes[:i]) for i in range(len(mb_sizes))]
    NMB = len(mb_sizes)
    # phase-2 tile -> (block, offset within block)
    mt_map = []
    for bi, (w, o) in enumerate(zip(mb_sizes, mb_offs)):
        for j in range(w // P):
            mt_map.append((bi, j * P))

    nc = bacc.Bacc(None, target_bir_lowering=False, debug=False)

    xT = nc.dram_tensor("xT", [P, KD, M], mm_dt, kind="ExternalInput")
    w1 = nc.dram_tensor("w1", [P, KD, H], mm_dt, kind="ExternalInput")
    w2 = nc.dram_tensor("w2", [P, KH, D], mm_dt, kind="ExternalInput")
    b1t = nc.dram_tensor("b1t", [P, NHC], f32, kind="ExternalInput")
    dsp = nc.dram_tensor("dsp", [P, NMT], f32, kind="ExternalInput")
    out = nc.dram_tensor("out", [P, NMT, D], f32, kind="ExternalOutput")

    with TileContext(nc) as tc:
        with tc.tile_pool(name="dram", bufs=1, space="DRAM") as dram, \
             tc.tile_pool(name="const", bufs=1) as const:
            # Intermediate hT, one DRAM tile per token block so phase 2
            # token tiles only depend on their own block's fc1 writes.
            hT_blocks = [
                dram.tile([P, NHC, mb_sizes[mb]], mm_dt, name=f"hT{mb}")
                for mb in range(NMB)
            ]
            b1_sb = const.tile([P, NHC], f32, name="b1_sb")
            nc.sync.dma_start(b1_sb[:], b1t[:])
            dsp_sb = const.tile([P, NMT], f32, name="dsp_sb")
            nc.sync.dma_start(dsp_sb[:], dsp[:])

            for rep in range(reps):
                # ---- Phase 1: hT = gelu(x @ W1 + b1), stored [H, M] ----
                with tc.tile_pool(name=f"w1p{rep}", bufs=1) as w1p, \
                     tc.tile_pool(name=f"xp{rep}", bufs=3) as xp, \
                     tc.tile_pool(name=f"hp{rep}", bufs=6) as hp, \
                     tc.tile_pool(name=f"ps1{rep}", bufs=4, space="PSUM") as ps1:
                    # First x block issued before the W1 chunk loads so the
                    # first matmul isn't queued behind 16MB of weight DMA.
                    x_first = xp.tile(
                        [P, KD, mb_sizes[0]], mm_dt, name="x_sb",
                        tag="x_sb")
                    nc.sync.dma_start(x_first[:], xT[:, :, 0:mb_sizes[0]])
                    # W1 resident, split per h-chunk for fine-grained deps.
                    w1_sb = []
                    for hc in range(NHC):
                        t = w1p.tile([P, KD, P], mm_dt, name=f"w1c{hc}")
                        nc.sync.dma_start(t[:], w1[:, :, hc * P:(hc + 1) * P])
                        w1_sb.append(t)
                    for mb in range(NMB):
                        w = mb_sizes[mb]
                        o = mb_offs[mb]
                        if mb == 0:
                            x_sb = x_first
                        else:
                            x_sb = xp.tile(
                                [P, KD, w], mm_dt, name="x_sb", tag="x_sb")
                            nc.sync.dma_start(x_sb[:], xT[:, :, o:o + w])
                        for hc in range(NHC):
                            psum = ps1.tile([P, w], f32, name="ps1t",
                                            tag="ps1t")
                            for k in range(KD):
                                nc.tensor.matmul(
                                    psum[:],
                                    lhsT=w1_sb[hc][:, k:k + 1, :],
                                    rhs=x_sb[:, k:k + 1, :],
                                    start=(k == 0),
                                    stop=(k == KD - 1),
                                )
                            h_sb = hp.tile([P, w], mm_dt, name="h_sb",
                                           tag="h_sb")
                            nc.scalar.activation(
                                h_sb[:], psum[:],
                                mybir.ActivationFunctionType.Gelu,
                                bias=b1_sb[:, hc:hc + 1],
                            )
                            nc.sync.dma_start(hT_blocks[mb][:, hc, :], h_sb[:])

                # ---- Phase 2: out = (hT.T @ W2) * disp ----
                with tc.tile_pool(name=f"w2p{rep}", bufs=1) as w2p, \
                     tc.tile_pool(name=f"hp2{rep}", bufs=3) as hp2, \
                     tc.tile_pool(name=f"op{rep}", bufs=6) as op, \
                     tc.tile_pool(name=f"ps2{rep}", bufs=4, space="PSUM") as ps2:
                    # First stationary tile issued before the W2 chunk loads
                    # so fc2's first matmul isn't queued behind 16MB of W2.
                    hT_first = hp2.tile([P, KH, P], mm_dt, name="hT_sb",
                                        tag="hT_sb")
                    nc.sync.dma_start(hT_first[:], hT_blocks[0][:, :, 0:P])
                    w2_sb = []
                    for k in range(KH):
                        t = w2p.tile([P, 1, D], mm_dt, name=f"w2c{k}")
                        nc.sync.dma_start(t[:], w2[:, k:k + 1, :])
                        w2_sb.append(t)
                    for mt in range(NMT):
                        mb, off = mt_map[mt]
                        if mt == 0:
                            hT_sb = hT_first
                        else:
                            hT_sb = hp2.tile([P, KH, P], mm_dt, name="hT_sb",
                                             tag="hT_sb")
                            nc.sync.dma_start(
                                hT_sb[:], hT_blocks[mb][:, :, off:off + P])
                        for db in range(NDB):
                            psum = ps2.tile([P, 512], f32, name="ps2t")
                            for k in range(KH):
                                nc.tensor.matmul(
                                    psum[:],
                                    lhsT=hT_sb[:, k:k + 1, :],
                                    rhs=w2_sb[k][:, :, db * 512:(db + 1) * 512],
                                    start=(k == 0),
                                    stop=(k == KH - 1),
                                )
                            o_sb = op.tile([P, 512], f32, name="o_sb")
                            nc.vector.tensor_scalar_mul(
                                o_sb[:], psum[:], dsp_sb[:, mt:mt + 1])
                            nc.sync.dma_start(
                                out[:, mt, db * 512:(db + 1) * 512], o_sb[:])

    nc.compile()
    return nc


def _build_fused(mm_dt_name, M=CAP, reps=1):
    """Single-pass fused FFN: fc1+gelu+fc2 per token block, hT stays in
    SBUF (no DRAM round-trip). Requires a 16-bit matmul dtype so both
    weight matrices fit in SBUF (64KB+64KB per partition)."""
    import concourse.bass as bass
    import concourse.bacc as bacc
    import concourse.mybir as mybir
    from concourse.tile import TileContext

    mm_dt = getattr(mybir.dt, mm_dt_name)
    f32 = mybir.dt.float32

    KD = D // P            # 8  d_k tiles (fc1 contraction)
    KH = H // P            # 32 h_k tiles (fc2 contraction)
    NHC = H // P           # 32 h chunks (fc1 output)
    BLK = 384              # token block: 2304 = 6*384, 384 = 3*128
    assert M % BLK == 0
    NB = M // BLK
    JT = BLK // P          # 3 token tiles per block (fc2)
    NMT = M // P           # 18
    NDB = D // 512         # 2
    WC1 = 512              # w1 h-chunk width (DMA granularity)
    NW1 = H // WC1         # 8
    KW2 = 4                # w2 k-tiles per DMA chunk
    NW2 = KH // KW2        # 8

    nc = bacc.Bacc(None, target_bir_lowering=False, debug=False)
    xT = nc.dram_tensor("xT", [P, KD, M], mm_dt, kind="ExternalInput")
    w1 = nc.dram_tensor("w1", [P, KD, H], mm_dt, kind="ExternalInput")
    w2 = nc.dram_tensor("w2", [P, KH, D], mm_dt, kind="ExternalInput")
    b1t = nc.dram_tensor("b1t", [P, NHC], f32, kind="ExternalInput")
    dsp = nc.dram_tensor("dsp", [P, NMT], f32, kind="ExternalInput")
    out = nc.dram_tensor("out", [P, NMT, D], f32, kind="ExternalOutput")

    with TileContext(nc) as tc:
        with tc.tile_pool(name="const", bufs=1) as const:
            b1_sb = const.tile([P, NHC], f32, name="b1_sb")
            nc.sync.dma_start(b1_sb[:], b1t[:])
            dsp_sb = const.tile([P, NMT], f32, name="dsp_sb")
            nc.sync.dma_start(dsp_sb[:], dsp[:])
            for rep in range(reps):
                with tc.tile_pool(name=f"w1p{rep}", bufs=1) as w1p, \
                     tc.tile_pool(name=f"w2p{rep}", bufs=1) as w2p, \
                     tc.tile_pool(name=f"xp{rep}", bufs=3) as xp, \
                     tc.tile_pool(name=f"hp{rep}", bufs=2) as hp, \
                     tc.tile_pool(name=f"op{rep}", bufs=4) as op, \
                     tc.tile_pool(name=f"ps1{rep}", bufs=3,
                                  space="PSUM") as ps1, \
                     tc.tile_pool(name=f"ps2{rep}", bufs=3,
                                  space="PSUM") as ps2:
                    x_tiles = {}

                    def load_x(b):
                        t = xp.tile([P, KD, BLK], mm_dt, name="x_sb",
                                    tag="x_sb")
                        nc.sync.dma_start(t[:], xT[:, :, b*BLK:(b+1)*BLK])
                        x_tiles[b] = t

                    # DMA issue order: x0, all W1 (fc1 of block 0 starts
                    # ~7us in), x1, all W2 (first needed when fc2(0) runs,
                    # after fc1(0) and fc1(1) = ~48us of PE runway).
                    load_x(0)
                    w1_sb = []
                    for c in range(NW1):
                        t = w1p.tile([P, KD, WC1], mm_dt, name=f"w1c{c}")
                        nc.sync.dma_start(t[:], w1[:, :, c*WC1:(c+1)*WC1])
                        w1_sb.append(t)
                    load_x(1)
                    w2_sb = []
                    for c in range(NW2):
                        t = w2p.tile([P, KW2, D], mm_dt, name=f"w2c{c}")
                        nc.sync.dma_start(t[:], w2[:, c*KW2:(c+1)*KW2, :])
                        w2_sb.append(t)

                    hT = {}

                    def fc1(b):
                        if b + 2 < NB and (b + 2) not in x_tiles:
                            load_x(b + 2)
                        x_sb = x_tiles[b]
                        t = hp.tile([P, NHC, BLK], mm_dt, name="hT",
                                    tag="hT")
                        for hc in range(NHC):
                            psum = ps1.tile([P, BLK], f32, name="ps1t",
                                            tag="ps1t")
                            c, off = hc // 4, (hc % 4) * P
                            for k in range(KD):
                                nc.tensor.matmul(
                                    psum[:],
                                    lhsT=w1_sb[c][:, k:k+1, off:off+P],
                                    rhs=x_sb[:, k:k+1, :],
                                    start=(k == 0),
                                    stop=(k == KD - 1))
                            nc.scalar.activation(
                                t[:, hc, :], psum[:],
                                mybir.ActivationFunctionType.Gelu,
                                bias=b1_sb[:, hc:hc+1])
                        hT[b] = t

                    def fc2(b):
                        t = hT.pop(b)
                        for j in range(JT):
                            mt = b * JT + j
                            for db in range(NDB):
                                psum = ps2.tile([P, 512], f32, name="ps2t",
                                                tag="ps2t")
                                for k in range(KH):
                                    c, kk = k // KW2, k % KW2
                                    nc.tensor.matmul(
                                        psum[:],
                                        lhsT=t[:, k:k+1, j*P:(j+1)*P],
                                        rhs=w2_sb[c][:, kk:kk+1,
                                                     db*512:(db+1)*512],
                                        start=(k == 0),
                                        stop=(k == KH - 1))
                                o_sb = op.tile([P, 512], f32, name="o_sb",
                                               tag="o_sb")
                                nc.vector.tensor_scalar_mul(
                                    o_sb[:], psum[:], dsp_sb[:, mt:mt+1])
                                nc.sync.dma_start(
                                    out[:, mt, db*512:(db+1)*512], o_sb[:])

                    # Software pipeline: fc1 runs one block ahead of fc2
                    # so fc2(0) starts ~48us in, after W2 has streamed in.
                    fc1(0)
                    for b in range(1, NB):
                        fc1(b)
                        fc2(b - 1)
                    fc2(NB - 1)

    nc.compile()
    return nc


def _np_mm_dtype(mm_dt_name):
    if mm_dt_name == "float16":
        return np.float16
    if mm_dt_name == "bfloat16":
        import ml_dtypes
        return ml_dtypes.bfloat16
    return np.float32


def _is_fused(mm_dt_name, m_tokens):
    return mm_dt_name in ("float16", "bfloat16") and m_tokens % 384 == 0


def _build_any(mm_dt_name, m_tokens, reps=1):
    if _is_fused(mm_dt_name, m_tokens):
        return _build_fused(mm_dt_name, M=m_tokens, reps=reps)
    return _build_nc(mm_dt_name, M=m_tokens, reps=reps)


def _get_nc(m_tokens=M):
    mm_dt_name = os.environ.get("KERNEL_MM_DT", "float16")
    if m_tokens != CAP and _is_fused(mm_dt_name, CAP) \
            and not _is_fused(mm_dt_name, m_tokens):
        mm_dt_name = "float32r"  # dense fallback: fused needs M%384==0
    key = ("nc", mm_dt_name, m_tokens)
    if key not in _CACHE:
        _CACHE[key] = _build_any(mm_dt_name, m_tokens)
    return _CACHE[key], mm_dt_name


class _Runner:
    """Cached jitted sharded invocation for one compiled Bass program."""

    def __init__(self, nc, n_cores):
        import jax
        from jax.sharding import Mesh, PartitionSpec
        from jax.experimental.shard_map import shard_map
        import concourse.mybir as mybir
        from concourse import bass2jax
        from concourse.bass2jax import _bass_exec_p, install_neuronx_cc_hook

        install_neuronx_cc_hook()
        self.jax = jax
        self.n_cores = n_cores
        partition_name = (
            nc.partition_id_tensor.name if nc.partition_id_tensor else None)
        in_names, out_names, out_avals = [], [], []
        for alloc in nc.m.functions[0].allocations:
            if not isinstance(alloc, mybir.MemoryLocationSet):
                continue
            name = alloc.memorylocations[0].name
            if alloc.kind == "ExternalInput":
                if name != partition_name:
                    in_names.append(name)
            elif alloc.kind == "ExternalOutput":
                out_names.append(name)
                out_avals.append(jax.core.ShapedArray(
                    tuple(alloc.tensor_shape), mybir.dt.np(alloc.dtype)))
        self.in_names = in_names
        self.out_names = out_names
        self.out_avals = out_avals
        n_params = len(in_names)
        n_outs = len(out_avals)
        all_in_names = in_names + out_names
        if partition_name is not None:
            all_in_names = all_in_names + [partition_name]

        def _body(*args):
            operands = list(args)
            if partition_name is not None:
                operands.append(bass2jax.partition_id_tensor())
            outs = _bass_exec_p.bind(
                *operands,
                out_avals=tuple(out_avals),
                in_names=tuple(all_in_names),
                out_names=tuple(out_names),
                lowering_input_output_aliases=(),
                sim_require_finite=True,
                sim_require_nnan=True,
                nc=nc,
            )
            return tuple(outs)

        devices = jax.devices()[:n_cores]
        mesh = Mesh(np.asarray(devices), ("core",))
        self.sh = jax.sharding.NamedSharding(mesh, PartitionSpec("core"))
        self.sharded = jax.jit(
            shard_map(_body, mesh=mesh,
                      in_specs=(PartitionSpec("core"),) * (n_params + n_outs),
                      out_specs=(PartitionSpec("core"),) * n_outs,
                      check_rep=False),
            donate_argnums=tuple(range(n_params, n_params + n_outs)),
            keep_unused=True)

    def put_inputs(self, in_maps):
        return [
            self.jax.device_put(
                np.concatenate(
                    [np.asarray(m[name]) for m in in_maps], axis=0), self.sh)
            for name in self.in_names
        ]

    def zeros(self):
        return [
            self.jax.device_put(
                np.zeros((self.n_cores * a.shape[0], *a.shape[1:]), a.dtype),
                self.sh)
            for a in self.out_avals
        ]

    def run(self, dev_in):
        out = self.sharded(*dev_in, *self.zeros())
        self.jax.block_until_ready(out)
        return out

    def to_results(self, out):
        return [
            {name: np.asarray(out[i]).reshape(
                self.n_cores, *self.out_avals[i].shape)[c]
             for i, name in enumerate(self.out_names)}
            for c in range(self.n_cores)
        ]


def _get_runner(nc):
    key = ("runner", id(nc))
    if key not in _CACHE:
        _CACHE[key] = _Runner(nc, NCORES)
    return _CACHE[key]


def bench_spmd(nc, in_maps, iters=5):
    """Time repeated on-device executions with device-resident inputs.
    Returns (best_seconds, results_of_last_call)."""
    import time as _time
    r = _get_runner(nc)
    dev_in = r.put_inputs(in_maps)
    out = r.run(dev_in)  # warmup (compiles once)
    best = float("inf")
    for _ in range(iters):
        z = r.zeros()
        r.jax.block_until_ready(z)
        t0 = _time.perf_counter()
        out = r.sharded(*dev_in, *z)
        r.jax.block_until_ready(out)
        best = min(best, _time.perf_counter() - t0)
    return best, r.to_results(out)


def _core_weight_inputs(W1, b1, W2, e, mdt=np.float32):
    return {
        "w1": _pm(W1[e].astype(mdt)),           # [128, 8, 4096]
        "w2": _pm(W2[e].astype(mdt)),           # [128, 32, 1024]
        "b1t": np.ascontiguousarray(
            b1[e].reshape(H // P, P).T),        # [128, 32]
    }


def _dense_in_maps(x2, disp, W1, b1, W2, mdt=np.float32):
    xT_pm = _pm(np.ascontiguousarray(x2.T).astype(mdt))  # [128, 8, 8192]
    in_maps = []
    for e in range(NCORES):
        m = _core_weight_inputs(W1, b1, W2, e, mdt)
        m["xT"] = xT_pm
        m["dsp"] = np.ascontiguousarray(disp[:, e].reshape(M // P, P).T)
        in_maps.append(m)
    return in_maps


def _sparse_in_maps(x2, disp, W1, b1, W2, mdt=np.float32):
    """Gather each expert's routed tokens (padded to CAP). Returns
    (in_maps, idx_list) or None if any expert overflows CAP."""
    in_maps, idx_list = [], []
    for e in range(NCORES):
        idx = np.nonzero(disp[:, e] > 0)[0]
        if idx.size > CAP:
            return None
        x_e = np.zeros((CAP, D), dtype=np.float32)
        x_e[:idx.size] = x2[idx]
        d_e = np.zeros((CAP,), dtype=np.float32)
        d_e[:idx.size] = disp[idx, e]
        m = _core_weight_inputs(W1, b1, W2, e, mdt)
        m["xT"] = _pm(np.ascontiguousarray(x_e.T).astype(mdt))
        m["dsp"] = np.ascontiguousarray(d_e.reshape(CAP // P, P).T)
        in_maps.append(m)
        idx_list.append(idx)
    return in_maps, idx_list


def _run_spmd(nc, in_maps):
    r = _get_runner(nc)
    out = r.run(r.put_inputs(in_maps))
    return r.to_results(out)


def kernel(x, Wr, W1, b1, W2, b2):
    global LAST_RESULTS

    x2 = np.ascontiguousarray(np.asarray(x, dtype=np.float32).reshape(M, D))
    Wr = np.asarray(Wr, dtype=np.float32)
    W1 = np.asarray(W1, dtype=np.float32)
    b1 = np.asarray(b1, dtype=np.float32)
    W2 = np.asarray(W2, dtype=np.float32)
    b2 = np.asarray(b2, dtype=np.float32)

    disp = _route_host(x2, Wr)  # [M, E]
    mode = os.environ.get("KERNEL_MODE", "auto")

    sparse = None
    if mode in ("auto", "sparse"):
        nc, mm_dt_name = _get_nc(CAP)
        sparse = _sparse_in_maps(x2, disp, W1, b1, W2,
                                 _np_mm_dtype(mm_dt_name))
    if sparse is not None:
        in_maps, idx_list = sparse
        results = _run_spmd(nc, in_maps)
        LAST_RESULTS = results
        out2 = np.zeros((M, D), dtype=np.float32)
        for e in range(NCORES):
            y = results[e]["out"].transpose(1, 0, 2).reshape(CAP, D)
            out2[idx_list[e]] += y[:idx_list[e].size]
    else:
        nc, mm_dt_name = _get_nc(M)
        in_maps = _dense_in_maps(x2, disp, W1, b1, W2,
                                 _np_mm_dtype(mm_dt_name))
        results = _run_spmd(nc, in_maps)
        LAST_RESULTS = results
        acc = np.zeros((P, M // P, D), dtype=np.float32)
        for r in results:
            acc += r["out"]
        out2 = acc.transpose(1, 0, 2).reshape(M, D)

    out2 = out2 + disp @ b2  # sum_e disp_e * b2[e]
    return out2.reshape(B, T, D)

